# revision 17
# baseline (speedup 1.0000x reference)
"""Trainium2 Bass kernel for nn_Net_84052509983223.

A 3-layer invertible-flow net: per layer [actnorm -> scaled circular 3x3 conv
+ bias -> slog gate], plus a per-batch logdet accumulator.

Strategy (pure data parallel over batch, 8 cores x 64 images):
  * The circular conv is computed on the TensorEngine as a sum of 9 matmuls
    per output channel with 128x128 circulant ("y-conv") matrices as the
    stationary operand; the x-shift of each tap is absorbed into the rhs
    access pattern using 2 halo columns kept per image row block.
  * actnorm is folded into the conv kernel (scale) and a constant output
    offset (bias); the identity part of the near-identity kernel is split
    off and added exactly in fp32 on the vector engine, so the PE only
    computes a small residual (fp32r precision is ample for it).
  * exp() of all the tiny/host-size parameters, the fused bias maps and the
    spectral logdet constant are precomputed on the host in float64.
  * slog gate runs on ACT (Abs, Ln=log1p) + DVE (sign transfer via bitwise
    ops); the per-batch logdet reduction rides for free on the Ln pass via
    accum_out, finished with a ones-matmul over partitions.

Layout: SBUF h-state is [y=128 partitions, (img, ch, 1+x+1)] with circular
x-halo columns; y-wrap is built into the circulant weights.
"""

import os
import sys

import numpy as np

F16 = np.dtype(np.float16)

for _p in ("/opt/trn_rl_repo",):
    if _p not in sys.path and os.path.isdir(_p):
        sys.path.insert(0, _p)

from concourse import bacc, mybir  # noqa: E402
from concourse import tile as tile_mod  # noqa: E402
from concourse.bass_utils import run_bass_kernel_spmd  # noqa: E402

F32 = mybir.dt.float32
HF = mybir.dt.float16
U32 = mybir.dt.uint32
AF = mybir.ActivationFunctionType
ALU = mybir.AluOpType

L = 3          # layers
C = 3          # channels
NN = 128       # image height/width
KK = 3         # conv kernel size
B = 512        # full batch
NCORES = 8
BC = B // NCORES   # images per core
CHUNK = 4          # images per matmul chunk (N = CHUNK*NN = 512)
NCHUNK = BC // CHUNK
BLOCK = 2          # chunks per PSUM round (3 co * BLOCK psum tiles live)
COLS = NN + 2      # x columns incl. circular halo; col i holds x=i-1


# ---------------------------------------------------------------------------
# host-side precomputation (float64)
# ---------------------------------------------------------------------------

def _host_consts(actnorm_bias, actnorm_log_scale, conv_kernel, conv_bias,
                 conv_log_scale, slog_log_alpha):
    a = np.exp(actnorm_log_scale.astype(np.float64))[:, 0, :, 0, 0]   # (L, C)
    b = actnorm_bias.astype(np.float64)[:, 0, :, 0, 0]                # (L, C)
    Kk = conv_kernel.astype(np.float64)                               # (L,C,C,3,3)
    e = np.exp(conv_log_scale.astype(np.float64))[:, 0]               # (L,C,N,N)
    cb = conv_bias.astype(np.float64)[:, 0]                           # (L,C,N,N)
    alpha = np.exp(slog_log_alpha.astype(np.float64))[:, 0, :, 0, 0]  # (L, C)

    # residual kernel R = K*a - I
    R = Kk * a[:, None, :, None, None]
    for l in range(L):
        for c in range(C):
            R[l, c, c, 1, 1] -= 1.0

    # constant conv output offset from the actnorm bias
    beta = np.einsum('loiyx,li->lo', Kk, b)
    cbf = e * beta[:, :, None, None] + cb     # fused bias map

    # circulant weights W[l, co, ci, dx][y, y'] with
    # out[y'] += sum_y W[y,y'] * h[(y), x'+dx-1],  y = (y'+dy-1) mod N
    W = np.zeros((L, C, C, 3, NN, NN), dtype=np.float64)
    yy = np.arange(NN)
    for dy in range(3):
        src = (yy + dy - 1) % NN
        for dx in range(3):
            W[:, :, :, dx, src, yy] = R[:, :, :, dy, dx][:, :, :, None]

    # scalar logdet constant
    ld_const = 0.0
    for l in range(L):
        ld_const += NN * NN * float(np.sum(np.log(a[l])))
        ld_const += float(np.sum(np.log(e[l])))
        Kp = np.zeros((C, C, NN, NN), dtype=np.float64)
        Kp[:, :, :KK, :KK] = Kk[l]
        M = np.transpose(np.fft.fft2(Kp), (2, 3, 0, 1))
        _, logabs = np.linalg.slogdet(M)
        ld_const += float(np.sum(logabs))

    # device tensors (conv weights in fp16 for the PE residual path)
    wts = np.ascontiguousarray(
        W.reshape(L, C * C * 3, NN, NN)).astype(F16)
    # maps tiled over CHUNK images: (L, y, co, CHUNK, x)
    emap = np.ascontiguousarray(
        np.broadcast_to(e.transpose(0, 2, 1, 3)[:, :, :, None, :],
                        (L, NN, C, CHUNK, NN))).astype(np.float32)
    cbmap = np.ascontiguousarray(
        np.broadcast_to(cbf.transpose(0, 2, 1, 3)[:, :, :, None, :],
                        (L, NN, C, CHUNK, NN))).astype(np.float32)
    acol = np.ascontiguousarray(
        np.broadcast_to(alpha.reshape(1, L * C), (NN, L * C))).astype(np.float32)
    iacol = np.ascontiguousarray(
        np.broadcast_to((1.0 / alpha).reshape(1, L * C),
                        (NN, L * C))).astype(np.float32)
    return dict(wts=wts, emap=emap, cbmap=cbmap, acol=acol, iacol=iacol,
                ld_const=ld_const)


# ---------------------------------------------------------------------------
# device program
# ---------------------------------------------------------------------------

def build_program(bc=BC):
    """Build the per-core Tile program. Identical on all cores (SPMD)."""
    nchunk = bc // CHUNK
    nc = bacc.Bacc("TRN2", target_bir_lowering=False, debug=False)

    x_in = nc.dram_tensor("x", (bc, C, NN, NN), F32, kind="ExternalInput")
    wts = nc.dram_tensor("wts", (L, C * C * 3, NN, NN), HF,
                         kind="ExternalInput")
    emap = nc.dram_tensor("emap", (L, NN, C, CHUNK, NN), F32,
                          kind="ExternalInput")
    cbmap = nc.dram_tensor("cbmap", (L, NN, C, CHUNK, NN), F32,
                           kind="ExternalInput")
    acol = nc.dram_tensor("acol", (NN, L * C), F32, kind="ExternalInput")
    iacol = nc.dram_tensor("iacol", (NN, L * C), F32, kind="ExternalInput")
    h_out = nc.dram_tensor("h_out", (bc, C, NN, NN), F32,
                           kind="ExternalOutput")
    ld_out = nc.dram_tensor("ld_out", (bc,), F32, kind="ExternalOutput")

    with tile_mod.TileContext(nc) as tc:
        with (
            tc.tile_pool(name="hpool", bufs=1) as hpool,
            tc.tile_pool(name="wpool", bufs=2) as wpool,
            tc.tile_pool(name="mpool", bufs=1) as mpool,
            tc.tile_pool(name="bpool", bufs=3) as bpool,
            tc.tile_pool(name="cpool", bufs=8) as cpool,
            tc.tile_pool(name="tpool", bufs=6) as tpool,
            tc.tile_pool(name="spool", bufs=4) as spool,
            tc.tile_pool(name="kpool", bufs=1) as kpool,
            tc.tile_pool(name="psum", bufs=7, space="PSUM") as ppool,
            tc.tile_pool(name="psum_ld", bufs=1, space="PSUM") as ppool_ld,
        ):
            # persistent state + constants
            h_t = hpool.tile([NN, bc, C, COLS], F32)
            ldacc = kpool.tile([NN, L * C, bc], F32, tag="ldacc")
            a_t = kpool.tile([NN, L * C], F32, tag="acol")
            ia_t = kpool.tile([NN, L * C], F32, tag="iacol")
            ones_t = kpool.tile([NN, 1], F32, tag="ones")

            nc.sync.dma_start(out=a_t[:], in_=acol[:])
            nc.sync.dma_start(out=ia_t[:], in_=iacol[:])
            nc.vector.memset(ones_t[:], 1.0)

            # load x into h state (partition = y), then fill halo columns
            for ch in range(nchunk):
                i0 = ch * CHUNK
                nc.sync.dma_start(
                    out=h_t[:, i0:i0 + CHUNK, :, 1:NN + 1],
                    in_=x_in[i0:i0 + CHUNK].rearrange("i c y x -> y i c x"),
                )
                nc.scalar.copy(h_t[:, i0:i0 + CHUNK, :, 0:1],
                               h_t[:, i0:i0 + CHUNK, :, NN:NN + 1])
                nc.scalar.copy(h_t[:, i0:i0 + CHUNK, :, NN + 1:NN + 2],
                               h_t[:, i0:i0 + CHUNK, :, 1:2])

            for l in range(L):
                # per-layer constants
                w_t = wpool.tile([NN, C * C * 3, NN], HF, tag="w")
                nc.sync.dma_start(out=w_t[:],
                                  in_=wts[l].rearrange("i y z -> y i z"))
                em_t = mpool.tile([NN, C, CHUNK, NN], F32, tag="em")
                cb_t = mpool.tile([NN, C, CHUNK, NN], F32, tag="cb")
                nc.sync.dma_start(out=em_t[:], in_=emap[l])
                nc.sync.dma_start(out=cb_t[:], in_=cbmap[l])

                for blk in range(0, nchunk, BLOCK):
                    chunks = range(blk, min(blk + BLOCK, nchunk))
                    b0 = blk * CHUNK
                    bn = len(chunks) * CHUNK
                    # fp16 shadow of this block's h (incl. halo) for the PE
                    hb = bpool.tile([NN, BLOCK * CHUNK, C, COLS], HF, tag="hb",
                                    name="hb")
                    nc.vector.tensor_copy(out=hb[:, :bn],
                                          in_=h_t[:, b0:b0 + bn])
                    # conv: accumulate 9 residual taps per (co, chunk)
                    ptiles = {}
                    for co in range(C):
                        for ch in chunks:
                            ptiles[(co, ch)] = ppool.tile(
                                [NN, CHUNK, NN], F32, tag="pt", name="pt")
                    for w in range(9):
                        ci, dx = divmod(w, 3)
                        for co in range(C):
                            lhsT = w_t[:, (co * 3 + ci) * 3 + dx, :]
                            for ch in chunks:
                                j0 = (ch - blk) * CHUNK
                                rhs = hb[:, j0:j0 + CHUNK, ci, dx:dx + NN]
                                nc.tensor.matmul(
                                    ptiles[(co, ch)][:],
                                    lhsT,
                                    rhs,
                                    start=(w == 0), stop=(w == 8),
                                )
                    # post-conv chain per (co, chunk)
                    for co in range(C):
                        aslc = a_t[:, l * C + co:l * C + co + 1]
                        iaslc = ia_t[:, l * C + co:l * C + co + 1]
                        for ch in chunks:
                            i0 = ch * CHUNK
                            hin = h_t[:, i0:i0 + CHUNK, co, 1:NN + 1]
                            c_t = cpool.tile([NN, CHUNK, NN], F32, tag="c")
                            # c = P + h  (exact identity part, fp32)
                            nc.vector.tensor_add(c_t[:], ptiles[(co, ch)][:],
                                                 hin)
                            # c = c * e + cb'
                            nc.vector.tensor_mul(c_t[:], c_t[:], em_t[:, co])
                            nc.vector.tensor_add(c_t[:], c_t[:], cb_t[:, co])
                            # sign bits of c
                            sb_t = spool.tile([NN, CHUNK, NN], U32, tag="sb")
                            nc.vector.tensor_scalar(
                                sb_t[:], c_t[:].bitcast(U32),
                                0x80000000, None, op0=ALU.bitwise_and)
                            # u = |alpha * c| ; v = log1p(u)
                            u_t = tpool.tile([NN, CHUNK, NN], F32, tag="u")
                            nc.scalar.activation(u_t[:], c_t[:], AF.Abs,
                                                 scale=aslc)
                            v_t = tpool.tile([NN, CHUNK, NN], F32, tag="v")
                            for j in range(CHUNK):
                                nc.scalar.activation(
                                    v_t[:, j], u_t[:, j], AF.Ln, bias=1.0,
                                    accum_out=ldacc[:, l * C + co,
                                                    i0 + j:i0 + j + 1])
                            # h = sign(c) * v / alpha
                            nc.vector.tensor_scalar_mul(v_t[:], v_t[:], iaslc)
                            nc.vector.tensor_tensor(
                                out=hin.bitcast(U32), in0=v_t[:].bitcast(U32),
                                in1=sb_t[:], op=ALU.bitwise_or)
                    # refresh halo columns / write output
                    for ch in chunks:
                        i0 = ch * CHUNK
                        if l < L - 1:
                            nc.scalar.copy(h_t[:, i0:i0 + CHUNK, :, 0:1],
                                           h_t[:, i0:i0 + CHUNK, :, NN:NN + 1])
                            nc.scalar.copy(
                                h_t[:, i0:i0 + CHUNK, :, NN + 1:NN + 2],
                                h_t[:, i0:i0 + CHUNK, :, 1:2])
                        else:
                            nc.sync.dma_start(
                                out=h_out[i0:i0 + CHUNK].rearrange(
                                    "i c y x -> y i c x"),
                                in_=h_t[:, i0:i0 + CHUNK, :, 1:NN + 1],
                            )

            # logdet: sum the 9 (layer,ch) accum columns, then reduce over
            # partitions with a ones-matmul
            red = kpool.tile([NN, bc], F32, tag="red")
            nc.vector.tensor_add(red[:], ldacc[:, 0], ldacc[:, 1])
            for k in range(2, L * C):
                nc.vector.tensor_add(red[:], red[:], ldacc[:, k])
            ld_ps = ppool_ld.tile([bc, 1], F32)
            nc.tensor.matmul(ld_ps[:], red[:], ones_t[:],
                             start=True, stop=True)
            ld_sb = kpool.tile([bc, 1], F32, tag="ldout")
            nc.scalar.copy(ld_sb[:], ld_ps[:])
            nc.sync.dma_start(out=ld_out[:], in_=ld_sb[:, 0])

    nc.compile()
    return nc


# ---------------------------------------------------------------------------
# public entry point
# ---------------------------------------------------------------------------

_PROGRAM_CACHE = {}
LAST_RESULTS = None  # BassKernelResults of the most recent run (for test.py)


def _get_program(bc=BC):
    if bc not in _PROGRAM_CACHE:
        _PROGRAM_CACHE[bc] = build_program(bc)
    return _PROGRAM_CACHE[bc]


def kernel(x, actnorm_bias, actnorm_log_scale, conv_kernel, conv_bias,
           conv_log_scale, slog_log_alpha):
    x = np.ascontiguousarray(np.asarray(x, dtype=np.float32))
    consts = _host_consts(np.asarray(actnorm_bias),
                          np.asarray(actnorm_log_scale),
                          np.asarray(conv_kernel), np.asarray(conv_bias),
                          np.asarray(conv_log_scale),
                          np.asarray(slog_log_alpha))

    nc = _get_program(BC)
    shared = {k: consts[k] for k in ("wts", "emap", "cbmap", "acol", "iacol")}
    in_maps = [
        dict(x=np.ascontiguousarray(x[c * BC:(c + 1) * BC]), **shared)
        for c in range(NCORES)
    ]
    global LAST_RESULTS
    LAST_RESULTS = run_bass_kernel_spmd(nc, in_maps, list(range(NCORES)))
    res = LAST_RESULTS.results

    h = np.concatenate([r["h_out"] for r in res], axis=0)
    vsum = np.concatenate([r["ld_out"] for r in res], axis=0)
    ld = (consts["ld_const"] - vsum.astype(np.float64)).astype(np.float32)
    return h, ld
